# revision 2
# baseline (speedup 1.0000x reference)
"""Trainium2 Bass kernel for nn_ClueCausalityExtractionThesis.

Single device phase, B=16 sharded 2 sentences/core across 8 NeuronCores,
all matmuls in bf16 (PSUM accumulates fp32):

  Host pre:  attention scores come from emb @ (Wg_w.T @ alpha) directly, so
             the masked softmax a[B,T,K] and the dense A.T matrix are built
             on host before launch. The GAT bias term b * has_child and the
             residual emb are folded into the uploaded embT_plus. A 32-column
             selection (last 32 clue positions) of A.T lets the device hand
             back exactly the new_emb columns the GRU tail needs.
  Device:    Wg_lin.T = (emb @ Wg_w.T).T ; attnT = Wg_lin.T @ A.T ;
             newT = attnT + embT_plus ; h6 = Wc6 @ newT ;
             asel = Wg_lin.T @ ATsel.
  Host post: x_proj for the 32 selected positions, 32-step GRU (contraction
             makes the full 1025-step masked scan equal to its last 32 clue
             steps within f32), rank-6 h-correction, output assembly.
"""

import sys

sys.path.insert(0, "/opt/trn_rl_repo")

from contextlib import ExitStack

import numpy as np
import ml_dtypes
import concourse.mybir as mybir
from concourse import bacc
from concourse.tile import TileContext

F32 = mybir.dt.float32
BF16 = mybir.dt.bfloat16
BF16_NP = ml_dtypes.bfloat16

B_PER_CORE = 2
T = 1024
D = 768
K = 8
H = 384
H3 = 3 * H
NW = 8   # T / 128
NE = 6   # D / 128
S_GRU = 32


def build_prog():
    nc = bacc.Bacc("TRN2", target_bir_lowering=False, debug=False)
    embT = nc.dram_tensor("embT", [B_PER_CORE, NE, 128, T], BF16, kind="ExternalInput").ap()
    atT = nc.dram_tensor("atT", [B_PER_CORE, NW, 128, T], BF16, kind="ExternalInput").ap()
    aselT = nc.dram_tensor("aselT", [B_PER_CORE, NW, 128, S_GRU], BF16, kind="ExternalInput").ap()
    wgwT = nc.dram_tensor("wgwT", [NE, 128, D], BF16, kind="ExternalInput").ap()
    wc6T = nc.dram_tensor("wc6T", [NE, 128, 8], BF16, kind="ExternalInput").ap()
    h6_o = nc.dram_tensor("h6_o", [B_PER_CORE, 8, T], F32, kind="ExternalOutput").ap()
    asel_o = nc.dram_tensor("asel_o", [B_PER_CORE, NE, 128, S_GRU], F32, kind="ExternalOutput").ap()

    with TileContext(nc) as tc, ExitStack() as ex:
        P = ex.enter_context
        const = P(tc.tile_pool(name="const", bufs=1))
        s_emb = P(tc.tile_pool(name="s_emb", bufs=2))
        s_at = P(tc.tile_pool(name="s_at", bufs=2))
        s_asl = P(tc.tile_pool(name="s_asl", bufs=2))
        s_wgt = P(tc.tile_pool(name="s_wgt", bufs=2))
        s_new = P(tc.tile_pool(name="s_new", bufs=2))
        s_out = P(tc.tile_pool(name="s_out", bufs=2))
        ps_wg = P(tc.tile_pool(name="ps_wg", bufs=2, space="PSUM"))
        ps_at = P(tc.tile_pool(name="ps_at", bufs=2, space="PSUM"))
        ps_hd = P(tc.tile_pool(name="ps_hd", bufs=2, space="PSUM"))

        wgwT_s = const.tile([128, NE, D], BF16, tag="wgwT")
        nc.sync.dma_start(out=wgwT_s[:], in_=wgwT.rearrange("a p b -> p a b"))
        wc6T_s = const.tile([128, NE, 8], BF16, tag="wc6T")
        nc.sync.dma_start(out=wc6T_s[:], in_=wc6T.rearrange("a p b -> p a b"))

        for b in range(B_PER_CORE):
            es = s_emb.tile([128, NE, T], BF16, tag="emb")
            nc.sync.dma_start(out=es[:], in_=embT[b].rearrange("a p b -> p a b"))
            ats = s_at.tile([128, NW, T], BF16, tag="at")
            nc.sync.dma_start(out=ats[:], in_=atT[b].rearrange("a p b -> p a b"))
            asls = s_asl.tile([128, NW, S_GRU], BF16, tag="asl")
            nc.sync.dma_start(out=asls[:], in_=aselT[b].rearrange("a p b -> p a b"))

            # ---- Wg_lin = emb @ Wg_w.T (no bias), cast to bf16 ----
            wgt = s_wgt.tile([128, NW, D], BF16, tag="wgt")
            for w in range(NW):
                pt = ps_wg.tile([128, D], F32, tag="wg")
                for n0, nn in ((0, 512), (512, 256)):
                    for ec in range(NE):
                        nc.tensor.matmul(
                            pt[:, n0 : n0 + nn],
                            es[:, ec, 128 * w : 128 * (w + 1)],
                            wgwT_s[:, ec, n0 : n0 + nn],
                            start=(ec == 0), stop=(ec == NE - 1),
                        )
                if w % 2 == 0:
                    nc.scalar.copy(wgt[:, w, :], pt[:])
                else:
                    nc.vector.tensor_copy(wgt[:, w, :], pt[:])

            # ---- attnT = Wg_lin.T @ A.T ; newT = attnT + embT_plus ----
            newT = s_new.tile([128, NE, T], BF16, tag="new")
            for m in range(NE):
                for th in range(2):
                    pj = ps_at.tile([128, 512], F32, tag="at")
                    for cc in range(NW):
                        nc.tensor.matmul(
                            pj[:],
                            wgt[:, cc, 128 * m : 128 * (m + 1)],
                            ats[:, cc, 512 * th : 512 * (th + 1)],
                            start=(cc == 0), stop=(cc == NW - 1),
                        )
                    nc.vector.tensor_add(
                        newT[:, m, 512 * th : 512 * (th + 1)],
                        pj[:],
                        es[:, m, 512 * th : 512 * (th + 1)],
                    )
                # ---- asel = Wg_lin.T @ ATsel (same stationary operands) ----
                pv = ps_at.tile([128, 512], F32, tag="at")
                for cc in range(NW):
                    nc.tensor.matmul(
                        pv[:, 0:S_GRU],
                        wgt[:, cc, 128 * m : 128 * (m + 1)],
                        asls[:, cc, :],
                        start=(cc == 0), stop=(cc == NW - 1),
                    )
                ao = s_out.tile([128, S_GRU], F32, tag="ao")
                nc.scalar.copy(ao[:], pv[:, 0:S_GRU])
                nc.sync.dma_start(out=asel_o[b, m], in_=ao[:])

            # ---- h6 = Wc6 @ newT ----
            hb = s_out.tile([8, T], F32, tag="h6")
            for th in range(2):
                ph = ps_hd.tile([8, 512], F32, tag="hd")
                for ec in range(NE):
                    nc.tensor.matmul(
                        ph[:],
                        wc6T_s[:, ec, :],
                        newT[:, ec, 512 * th : 512 * (th + 1)],
                        start=(ec == 0), stop=(ec == NE - 1),
                    )
                nc.scalar.copy(hb[:, 512 * th : 512 * (th + 1)], ph[:])
            nc.sync.dma_start(out=h6_o[b], in_=hb[:])
    nc.compile()
    return nc


_PROG = None


def _get_prog():
    global _PROG
    if _PROG is None:
        _PROG = build_prog()
    return _PROG


def _leaky_relu(x):
    return np.where(x > 0, x, np.float32(0.2) * x)


def host_prep(inputs):
    """All host-side preprocessing; returns per-core input maps."""
    emb = np.asarray(inputs["emb"], np.float32)
    Wg_w = np.asarray(inputs["Wg_w"], np.float32)
    Wg_b = np.asarray(inputs["Wg_b"], np.float32)
    al = np.asarray(inputs["alpha_left"], np.float32)
    ar = np.asarray(inputs["alpha_right"], np.float32)
    Wc_w = np.asarray(inputs["Wc_w"], np.float32)
    We_w = np.asarray(inputs["We_w"], np.float32)
    child_idx = np.asarray(inputs["child_idx"]).astype(np.int64)
    child_mask = np.asarray(inputs["child_mask"]).astype(np.int64)
    clue_mask = np.asarray(inputs["clue_mask"]).astype(np.int64)
    B = emb.shape[0]
    n_cores = B // B_PER_CORE

    # attention scores straight from emb (no device round trip needed)
    v2 = np.stack([Wg_w.T @ ar, Wg_w.T @ al], 1)              # [D, 2]
    sco = emb.reshape(-1, D) @ v2                              # [B*T, 2]
    sco = sco.reshape(B, T, 2)
    right_score = sco[:, :, 0] + float(ar @ Wg_b)
    self_score = sco[:, :, 1] + float(al @ Wg_b)

    bi = np.arange(B)[:, None, None]
    child_score = right_score[bi, child_idx]
    mask = child_mask.astype(bool)
    s = _leaky_relu(self_score[..., None] + child_score).astype(np.float32)
    s = np.where(mask, s, np.float32(-1e9))
    s = s - s.max(-1, keepdims=True)
    e = np.exp(s, dtype=np.float32)
    a = e / e.sum(-1, keepdims=True)
    a = np.where(mask, a, 0.0).astype(np.float32)
    has_child = mask.any(-1)                                   # [B, T]

    # dense A.T  (AT[b, c, t] = sum_k a[b,t,k] [child_idx[b,t,k]==c])
    AT = np.zeros((B, T, T), np.float32)
    tt = np.broadcast_to(np.arange(T)[None, :, None], child_idx.shape)
    np.add.at(AT, (bi, child_idx, tt), a)
    AT_bf = AT.reshape(B, NW, 128, T).astype(BF16_NP)

    # embT_plus = emb.T + Wg_b x has_child  (GAT bias + residual folded in)
    embT_plus = emb.transpose(0, 2, 1) + Wg_b[None, :, None] * has_child[:, None, :]
    embT_bf = embT_plus.reshape(B, NE, 128, T).astype(BF16_NP)

    # last S_GRU clue positions per sentence (in [CLS; seq] space)
    m = np.concatenate([np.ones((B, 1), bool), clue_mask.astype(bool)], 1)
    sel_pos = np.zeros((B, S_GRU), np.int64)
    sel_cnt = np.zeros(B, np.int64)
    ATsel = np.zeros((B, T, S_GRU), np.float32)
    for b in range(B):
        pos = np.where(m[b])[0][-S_GRU:]
        sel_cnt[b] = len(pos)
        sel_pos[b, S_GRU - len(pos):] = pos
        for j, p in enumerate(pos):
            jj = S_GRU - len(pos) + j
            if p == 0:
                continue                                        # CLS row: host handles
            t = p - 1
            for k in range(K):
                if mask[b, t, k]:
                    ATsel[b, child_idx[b, t, k], jj] += a[b, t, k]
    ATsel_bf = ATsel.reshape(B, NW, 128, S_GRU).astype(BF16_NP)

    wgwT_bf = np.ascontiguousarray(Wg_w.T).reshape(NE, 128, D).astype(BF16_NP)
    wc6 = np.zeros((D, 8), np.float32)
    wc6[:, 0:3] = Wc_w[:, :D].T
    wc6[:, 3:6] = We_w[:, :D].T
    wc6T_bf = wc6.reshape(NE, 128, 8).astype(BF16_NP)

    maps = [
        dict(
            embT=embT_bf[c * B_PER_CORE : (c + 1) * B_PER_CORE],
            atT=AT_bf[c * B_PER_CORE : (c + 1) * B_PER_CORE],
            aselT=ATsel_bf[c * B_PER_CORE : (c + 1) * B_PER_CORE],
            wgwT=wgwT_bf,
            wc6T=wc6T_bf,
        )
        for c in range(n_cores)
    ]
    aux = dict(sel_pos=sel_pos, sel_cnt=sel_cnt, has_child=has_child)
    return maps, aux


def host_post(inputs, res, aux):
    emb = np.asarray(inputs["emb"], np.float32)
    cls_embed = np.asarray(inputs["cls_embed"], np.float32)
    Wg_b = np.asarray(inputs["Wg_b"], np.float32)
    Wih = np.asarray(inputs["gru_Wih"], np.float32)
    bih = np.asarray(inputs["gru_bih"], np.float32)
    Whh = np.asarray(inputs["gru_Whh"], np.float32)
    bhh = np.asarray(inputs["gru_bhh"], np.float32)
    Wc_w = np.asarray(inputs["Wc_w"], np.float32)
    Wc_b = np.asarray(inputs["Wc_b"], np.float32)
    We_w = np.asarray(inputs["We_w"], np.float32)
    We_b = np.asarray(inputs["We_b"], np.float32)
    B = emb.shape[0]
    sel_pos, sel_cnt, has_child = aux["sel_pos"], aux["sel_cnt"], aux["has_child"]

    heads6 = np.concatenate([r["h6_o"] for r in res])          # [B, 8, T]
    asel = np.concatenate([r["asel_o"] for r in res])          # [B, 6, 128, 32]
    asel = asel.reshape(B, D, S_GRU)

    # new_emb at selected positions: exact emb + bias fold + device attn part
    x_cls = cls_embed @ Wih.T + bih
    X = np.zeros((B, S_GRU, H3), np.float32)
    for b in range(B):
        j0 = S_GRU - sel_cnt[b]
        for j in range(j0, S_GRU):
            p = sel_pos[b, j]
            if p == 0:
                X[b, j] = x_cls
            else:
                t = p - 1
                g = emb[b, t] + Wg_b * has_child[b, t] + asel[b, :, j]
                X[b, j] = g @ Wih.T + bih

    h = np.zeros((B, H), np.float32)
    for j in range(S_GRU):
        live = (j >= (S_GRU - sel_cnt))[:, None]
        hp = h @ Whh.T + bhh
        xr, xz, xn = np.split(X[:, j], 3, -1)
        hr, hz, hn = np.split(hp, 3, -1)
        r = 1.0 / (1.0 + np.exp(-(xr + hr)))
        z = 1.0 / (1.0 + np.exp(-(xz + hz)))
        n = np.tanh(xn + r * hn)
        h_new = ((1.0 - z) * n + z * h).astype(np.float32)
        h = np.where(live, h_new, h)

    corr = np.concatenate(
        [h @ Wc_w[:, D:].T + Wc_b, h @ We_w[:, D:].T + We_b], 1
    )                                                           # [B, 6]
    O6 = heads6[:, 0:6, :] + corr[:, :, None]
    O_cause = np.ascontiguousarray(O6[:, 0:3, :].transpose(0, 2, 1))
    O_effect = np.ascontiguousarray(O6[:, 3:6, :].transpose(0, 2, 1))
    return O_cause, O_effect


def kernel(**inputs):
    from concourse.bass_utils import run_bass_kernel_spmd

    maps, aux = host_prep(inputs)
    prog = _get_prog()
    res = run_bass_kernel_spmd(prog, maps, list(range(len(maps)))).results
    return host_post(inputs, res, aux)


# revision 25
# speedup vs baseline: 1.1652x; 1.1652x over previous
"""Trainium2 Bass kernel for nn_ClueCausalityExtractionThesis.

Single device phase, B=16 sharded 2 sentences/core across 8 NeuronCores,
all matmuls in bf16 (PSUM accumulates fp32):

  Host pre:  attention scores come from emb @ (Wg_w.T @ alpha) directly, so
             the masked softmax a[B,T,K] and the dense A.T matrix are built
             on host before launch. The GAT bias term b * has_child and the
             residual emb are folded into the uploaded embT_plus. A 32-column
             selection (last 32 clue positions) of A.T lets the device hand
             back exactly the new_emb columns the GRU tail needs.
  Device:    Wg_lin.T = (emb @ Wg_w.T).T ; attnT = Wg_lin.T @ A.T ;
             newT = attnT + embT_plus ; h6 = Wc6 @ newT ;
             asel = Wg_lin.T @ ATsel.
  Host post: x_proj for the 32 selected positions, 32-step GRU (contraction
             makes the full 1025-step masked scan equal to its last 32 clue
             steps within f32), rank-6 h-correction, output assembly.
"""

import sys

sys.path.insert(0, "/opt/trn_rl_repo")

from contextlib import ExitStack

import numpy as np
import ml_dtypes
import concourse.mybir as mybir
from concourse import bacc
from concourse.tile import TileContext

F32 = mybir.dt.float32
BF16 = mybir.dt.bfloat16
BF16_NP = ml_dtypes.bfloat16

B_PER_CORE = 2
T = 1024
D = 768
K = 8
H = 384
H3 = 3 * H
NW = 8   # T / 128
NE = 6   # D / 128
S_GRU = 32


def build_prog():
    nc = bacc.Bacc("TRN2", target_bir_lowering=False, debug=False)
    embT = nc.dram_tensor("embT", [B_PER_CORE, NE, 128, T], BF16, kind="ExternalInput").ap()
    atT = nc.dram_tensor("atT", [B_PER_CORE, NW, 128, T], BF16, kind="ExternalInput").ap()
    aselT = nc.dram_tensor("aselT", [B_PER_CORE, NW, 128, S_GRU], BF16, kind="ExternalInput").ap()
    wgwT = nc.dram_tensor("wgwT", [NE, 128, D], BF16, kind="ExternalInput").ap()
    wc6T = nc.dram_tensor("wc6T", [NE, 128, 8], BF16, kind="ExternalInput").ap()
    h6_o = nc.dram_tensor("h6_o", [B_PER_CORE, 8, T], F32, kind="ExternalOutput").ap()
    asel_o = nc.dram_tensor("asel_o", [B_PER_CORE, NE, 128, S_GRU], F32, kind="ExternalOutput").ap()

    with TileContext(nc) as tc, ExitStack() as ex:
        P = ex.enter_context
        const = P(tc.tile_pool(name="const", bufs=1))
        s_emb = P(tc.tile_pool(name="s_emb", bufs=1))
        s_at = P(tc.tile_pool(name="s_at", bufs=1))
        s_asl = P(tc.tile_pool(name="s_asl", bufs=1))
        s_wgt = P(tc.tile_pool(name="s_wgt", bufs=1))
        s_new = P(tc.tile_pool(name="s_new", bufs=1))
        s_out = P(tc.tile_pool(name="s_out", bufs=6))
        ps_wg = P(tc.tile_pool(name="ps_wg", bufs=3, space="PSUM"))
        ps_at = P(tc.tile_pool(name="ps_at", bufs=3, space="PSUM"))
        ps_sm = P(tc.tile_pool(name="ps_sm", bufs=2, space="PSUM"))

        # critical path first: wgwT then emb quarters (sync queue); bulk A.T
        # and small aux tensors go on the scalar HWDGE queue.
        # DMA order: tiny first-needed pieces first so PE starts ~3us in.
        wgwT_s = const.tile([128, NE, D], BF16, tag="wgwT")
        wc6T_s = const.tile([128, NE, 8], BF16, tag="wc6T")
        es_l, ats_l, asls_l = [], [], []
        for b in range(B_PER_CORE):
            es_b = s_emb.tile([128, NE, T], BF16, tag=f"emb{b}")
            ats_b = s_at.tile([128, NW, T], BF16, tag=f"at{b}")
            asls_b = s_asl.tile([128, NW, S_GRU], BF16, tag=f"asl{b}")
            es_l.append(es_b)
            ats_l.append(ats_b)
            asls_l.append(asls_b)
        # Supply in demand order: es strip w0, wgw thirds, es strips w1..7,
        # then sentence 1. Model serializes DMA transfers, so granularity and
        # order set when each Wg-mm group can fire.
        nc.sync.dma_start(
            out=es_l[0][:, :, 0:128],
            in_=embT[0, :, :, 0:128].rearrange("a p b -> p a b"),
        )
        for n0, nn in ((512, 256), (256, 256), (0, 256)):
            nc.sync.dma_start(
                out=wgwT_s[:, :, n0 : n0 + nn],
                in_=wgwT[:, :, n0 : n0 + nn].rearrange("a p b -> p a b"),
            )
        for w in range(1, NW):
            nc.sync.dma_start(
                out=es_l[0][:, :, 128 * w : 128 * (w + 1)],
                in_=embT[0, :, :, 128 * w : 128 * (w + 1)].rearrange("a p b -> p a b"),
            )
        for q in range(4):
            q0, q1 = 256 * q, 256 * (q + 1)
            nc.sync.dma_start(
                out=es_l[1][:, :, q0:q1],
                in_=embT[1, :, :, q0:q1].rearrange("a p b -> p a b"),
            )
        for b in range(B_PER_CORE):
            nc.sync.dma_start(out=ats_l[b][:], in_=atT[b].rearrange("a p b -> p a b"))
            nc.sync.dma_start(out=asls_l[b][:], in_=aselT[b].rearrange("a p b -> p a b"))
        nc.sync.dma_start(out=wc6T_s[:], in_=wc6T.rearrange("a p b -> p a b"))

        # ---- Wg_lin = emb @ Wg_w.T (no bias), cast to bf16 ----
        wgt_l = []
        for b in range(B_PER_CORE):
            es = es_l[b]
            wgt_b = s_wgt.tile([128, NW, D], BF16, tag=f"wgt{b}")
            wgt_l.append(wgt_b)
            wgt = wgt_b
            for w in range(NW):
                for i, n0 in enumerate((512, 256, 0)):
                    pt = ps_wg.tile([128, 256], F32, tag="wg")
                    for ec in range(NE):
                        nc.tensor.matmul(
                            pt[:],
                            es[:, ec, 128 * w : 128 * (w + 1)],
                            wgwT_s[:, ec, n0 : n0 + 256],
                            start=(ec == 0), stop=(ec == NE - 1),
                        )
                    if (w + i) % 2 == 0:
                        nc.scalar.copy(wgt[:, w, n0 : n0 + 256], pt[:])
                    else:
                        nc.vector.tensor_copy(wgt[:, w, n0 : n0 + 256], pt[:])

        # ---- attnT = Wg_lin.T @ A.T ; newT = attnT + embT_plus ;
        #      h6 = Wc6 @ newT per t-half; asel groups fill PE stalls ----
        asel_n = [0]

        def asel_group(b, m):
            pv = ps_sm.tile([128, 512], F32, tag="sm")
            for cc in range(NW):
                nc.tensor.matmul(
                    pv[:, 0:S_GRU],
                    wgt_l[b][:, cc, 128 * m : 128 * (m + 1)],
                    asls_l[b][:, cc, :],
                    start=(cc == 0), stop=(cc == NW - 1),
                )
            ao = s_out.tile([128, S_GRU], F32, tag="ao")
            if asel_n[0] % 2 == 0:
                nc.scalar.copy(ao[:], pv[:, 0:S_GRU])
            else:
                nc.vector.tensor_copy(ao[:], pv[:, 0:S_GRU])
            asel_n[0] += 1
            nc.sync.dma_start(out=asel_o[b, m], in_=ao[:])

        def heads_group(b, th):
            hb = s_out.tile([8, 512], F32, tag="h6")
            ph = ps_sm.tile([128, 512], F32, tag="sm")
            for ec in range(NE):
                nc.tensor.matmul(
                    ph[0:8, :],
                    wc6T_s[:, ec, :],
                    newT_l[b][:, ec, 512 * th : 512 * (th + 1)],
                    start=(ec == 0), stop=(ec == NE - 1),
                )
            nc.scalar.copy(hb[:], ph[0:8, :])
            nc.sync.dma_start(out=h6_o[b, :, 512 * th : 512 * (th + 1)], in_=hb[:])

        newT_l = []
        for b in range(B_PER_CORE):
            newT_b = s_new.tile([128, NE, T], BF16, tag=f"new{b}")
            newT_l.append(newT_b)
        # heads(b, th) is emitted one A-group late so its wait on the m=5
        # residual add hides under the next A-group's PE work.
        pending = None
        for b in range(B_PER_CORE):
            es, ats, wgt = es_l[b], ats_l[b], wgt_l[b]
            newT = newT_l[b]
            for th in range(2):
                for m in range(NE):
                    pj = ps_at.tile([128, 512], F32, tag="at")
                    for cc in range(NW):
                        nc.tensor.matmul(
                            pj[:],
                            wgt[:, cc, 128 * m : 128 * (m + 1)],
                            ats[:, cc, 512 * th : 512 * (th + 1)],
                            start=(cc == 0), stop=(cc == NW - 1),
                        )
                    nc.vector.tensor_add(
                        newT[:, m, 512 * th : 512 * (th + 1)],
                        pj[:],
                        es[:, m, 512 * th : 512 * (th + 1)],
                    )
                    if m == 0 and pending is not None:
                        heads_group(*pending)
                        pending = None
                for m in (0, 1, 2) if th == 0 else (3, 4, 5):
                    asel_group(b, m)
                pending = (b, th)
        heads_group(*pending)
    nc.compile()
    return nc


_PROG = None


def _get_prog():
    global _PROG
    if _PROG is None:
        _PROG = build_prog()
    return _PROG


def _leaky_relu(x):
    return np.where(x > 0, x, np.float32(0.2) * x)


def host_prep(inputs):
    """All host-side preprocessing; returns per-core input maps."""
    emb = np.asarray(inputs["emb"], np.float32)
    Wg_w = np.asarray(inputs["Wg_w"], np.float32)
    Wg_b = np.asarray(inputs["Wg_b"], np.float32)
    al = np.asarray(inputs["alpha_left"], np.float32)
    ar = np.asarray(inputs["alpha_right"], np.float32)
    Wc_w = np.asarray(inputs["Wc_w"], np.float32)
    We_w = np.asarray(inputs["We_w"], np.float32)
    child_idx = np.asarray(inputs["child_idx"]).astype(np.int64)
    child_mask = np.asarray(inputs["child_mask"]).astype(np.int64)
    clue_mask = np.asarray(inputs["clue_mask"]).astype(np.int64)
    B = emb.shape[0]
    n_cores = B // B_PER_CORE

    # attention scores straight from emb (no device round trip needed)
    v2 = np.stack([Wg_w.T @ ar, Wg_w.T @ al], 1)              # [D, 2]
    sco = emb.reshape(-1, D) @ v2                              # [B*T, 2]
    sco = sco.reshape(B, T, 2)
    right_score = sco[:, :, 0] + float(ar @ Wg_b)
    self_score = sco[:, :, 1] + float(al @ Wg_b)

    bi = np.arange(B)[:, None, None]
    child_score = right_score[bi, child_idx]
    mask = child_mask.astype(bool)
    s = _leaky_relu(self_score[..., None] + child_score).astype(np.float32)
    s = np.where(mask, s, np.float32(-1e9))
    s = s - s.max(-1, keepdims=True)
    e = np.exp(s, dtype=np.float32)
    a = e / e.sum(-1, keepdims=True)
    a = np.where(mask, a, 0.0).astype(np.float32)
    has_child = mask.any(-1)                                   # [B, T]

    # dense A.T  (AT[b, c, t] = sum_k a[b,t,k] [child_idx[b,t,k]==c])
    AT = np.zeros((B, T, T), np.float32)
    tt = np.broadcast_to(np.arange(T)[None, :, None], child_idx.shape)
    np.add.at(AT, (bi, child_idx, tt), a)
    AT_bf = AT.reshape(B, NW, 128, T).astype(BF16_NP)

    # embT_plus = emb.T + Wg_b x has_child  (GAT bias + residual folded in)
    embT_plus = emb.transpose(0, 2, 1) + Wg_b[None, :, None] * has_child[:, None, :]
    embT_bf = embT_plus.reshape(B, NE, 128, T).astype(BF16_NP)

    # last S_GRU clue positions per sentence (in [CLS; seq] space)
    m = np.concatenate([np.ones((B, 1), bool), clue_mask.astype(bool)], 1)
    sel_pos = np.zeros((B, S_GRU), np.int64)
    sel_cnt = np.zeros(B, np.int64)
    ATsel = np.zeros((B, T, S_GRU), np.float32)
    for b in range(B):
        pos = np.where(m[b])[0][-S_GRU:]
        sel_cnt[b] = len(pos)
        sel_pos[b, S_GRU - len(pos):] = pos
        for j, p in enumerate(pos):
            jj = S_GRU - len(pos) + j
            if p == 0:
                continue                                        # CLS row: host handles
            t = p - 1
            for k in range(K):
                if mask[b, t, k]:
                    ATsel[b, child_idx[b, t, k], jj] += a[b, t, k]
    ATsel_bf = ATsel.reshape(B, NW, 128, S_GRU).astype(BF16_NP)

    wgwT_bf = np.ascontiguousarray(Wg_w.T).reshape(NE, 128, D).astype(BF16_NP)
    wc6 = np.zeros((D, 8), np.float32)
    wc6[:, 0:3] = Wc_w[:, :D].T
    wc6[:, 3:6] = We_w[:, :D].T
    wc6T_bf = wc6.reshape(NE, 128, 8).astype(BF16_NP)

    maps = [
        dict(
            embT=embT_bf[c * B_PER_CORE : (c + 1) * B_PER_CORE],
            atT=AT_bf[c * B_PER_CORE : (c + 1) * B_PER_CORE],
            aselT=ATsel_bf[c * B_PER_CORE : (c + 1) * B_PER_CORE],
            wgwT=wgwT_bf,
            wc6T=wc6T_bf,
        )
        for c in range(n_cores)
    ]
    aux = dict(sel_pos=sel_pos, sel_cnt=sel_cnt, has_child=has_child)
    return maps, aux


def host_post(inputs, res, aux):
    emb = np.asarray(inputs["emb"], np.float32)
    cls_embed = np.asarray(inputs["cls_embed"], np.float32)
    Wg_b = np.asarray(inputs["Wg_b"], np.float32)
    Wih = np.asarray(inputs["gru_Wih"], np.float32)
    bih = np.asarray(inputs["gru_bih"], np.float32)
    Whh = np.asarray(inputs["gru_Whh"], np.float32)
    bhh = np.asarray(inputs["gru_bhh"], np.float32)
    Wc_w = np.asarray(inputs["Wc_w"], np.float32)
    Wc_b = np.asarray(inputs["Wc_b"], np.float32)
    We_w = np.asarray(inputs["We_w"], np.float32)
    We_b = np.asarray(inputs["We_b"], np.float32)
    B = emb.shape[0]
    sel_pos, sel_cnt, has_child = aux["sel_pos"], aux["sel_cnt"], aux["has_child"]

    heads6 = np.concatenate([r["h6_o"] for r in res])          # [B, 8, T]
    asel = np.concatenate([r["asel_o"] for r in res])          # [B, 6, 128, 32]
    asel = asel.reshape(B, D, S_GRU)

    # new_emb at selected positions: exact emb + bias fold + device attn part
    x_cls = cls_embed @ Wih.T + bih
    X = np.zeros((B, S_GRU, H3), np.float32)
    for b in range(B):
        j0 = S_GRU - sel_cnt[b]
        for j in range(j0, S_GRU):
            p = sel_pos[b, j]
            if p == 0:
                X[b, j] = x_cls
            else:
                t = p - 1
                g = emb[b, t] + Wg_b * has_child[b, t] + asel[b, :, j]
                X[b, j] = g @ Wih.T + bih

    h = np.zeros((B, H), np.float32)
    for j in range(S_GRU):
        live = (j >= (S_GRU - sel_cnt))[:, None]
        hp = h @ Whh.T + bhh
        xr, xz, xn = np.split(X[:, j], 3, -1)
        hr, hz, hn = np.split(hp, 3, -1)
        r = 1.0 / (1.0 + np.exp(-(xr + hr)))
        z = 1.0 / (1.0 + np.exp(-(xz + hz)))
        n = np.tanh(xn + r * hn)
        h_new = ((1.0 - z) * n + z * h).astype(np.float32)
        h = np.where(live, h_new, h)

    corr = np.concatenate(
        [h @ Wc_w[:, D:].T + Wc_b, h @ We_w[:, D:].T + We_b], 1
    )                                                           # [B, 6]
    O6 = heads6[:, 0:6, :] + corr[:, :, None]
    O_cause = np.ascontiguousarray(O6[:, 0:3, :].transpose(0, 2, 1))
    O_effect = np.ascontiguousarray(O6[:, 3:6, :].transpose(0, 2, 1))
    return O_cause, O_effect


def kernel(**inputs):
    from concourse.bass_utils import run_bass_kernel_spmd

    maps, aux = host_prep(inputs)
    prog = _get_prog()
    res = run_bass_kernel_spmd(prog, maps, list(range(len(maps)))).results
    return host_post(inputs, res, aux)
